# revision 21
# baseline (speedup 1.0000x reference)
"""Causal multi-head self-attention on 8 Trainium2 NeuronCores.

Problem: x[4,2048,1024] fp32, Wq/Wk/Wv/Wo[1024,1024] fp32 (torch Linear
weights, applied as x @ W.T), 16 heads, causal softmax attention.

Sharding: data-parallel over batch (4) x tensor-parallel over heads (2
groups of 8). Core c handles batch c//2 and head-group c%2: Wq/Wk/Wv are
column-sharded (512 output dims per core), Wo row-sharded; each core
produces a partial [2048,1024] output (bf16) and the host sums the two
partials per batch in fp32 ("all-reduce" done in the unshard step).

Per-core schedule (v3 — ScalarE(exp)-paced, PE kept warm with fillers):
  - startup: input DMAs issued in parallel across engine queues (x block 0
    + Wq first so the first matmul can start ~10us in, not ~50us).
  - proj(sb2=0): Q^T,K^T,V chunks; PSUM->SBUF casts on ScalarE (idle then).
  - attention per query block qb (nkb=4(qb+1) causal key blocks), heads
    processed in pairs (2cc, 2cc+1) living on partition halves 0-63/64-127:
    score matmuls are K=64 row-packed (tile_position (0,0)/(64,0)) so both
    PE array halves run concurrently; exp on ScalarE (scale=1/8 fused, no
    max-subtraction: scores bounded ~|6.5|); causal mask multiply only on
    the affected query prefix; PV matmuls interleaved into the score-group
    loop; softmax denominator Z rides as a 65th V column (ones).
  - normalize: Z rows broadcast over partitions via two accumulating K=1
    f32r matmuls (indicator weights), then a hand-rolled fast reciprocal
    (magic-constant seed + one Newton step) on DVE yields -1/Z; the sign
    is folded into the host-negated Wo.
  - proj(sb2=1) + output-projection chunks are emitted as "filler" PE work
    interleaved into the (ScalarE-bound) attention pair loop, keeping the
    PE busy/warm while respecting data readiness (split at 512-col
    granularity so qb2/qb3 inputs arrive just in time).
"""

import os
import sys

from collections import deque

import numpy as np

if "/opt/trn_rl_repo" not in sys.path:
    sys.path.insert(0, "/opt/trn_rl_repo")

B, S, D = 4, 2048, 1024
H, HL, DK = 16, 8, 64  # total heads, local heads per core, head dim
C = HL * DK            # local projection width (512)
NCORES = 8

_built = None


def _patch_tile_drain():
    """walrus in this container rejects the TileContext exit drain when it
    carries >1 sync-wait; split the extra waits onto standalone NOPs."""
    import concourse.mybir as mybir
    import concourse.tile as tile
    from concourse.vector_clock import ScopedClock

    if getattr(tile.TileContext, "_drain_split_patched", False):
        return

    def _drain_and_barrier(self, tick_clock, wait_clock):
        nc = self.nc
        drain_inst = nc.sync.drain()
        wait_clock.add_sem_waits(
            drain_inst.ins, ScopedClock({None: tick_clock.global_clock})
        )
        si = drain_inst.ins.sync_info
        if si is not None and si.on_wait and len(si.on_wait) > 1:
            waits = list(si.on_wait)
            si.on_wait = waits[:1]
            for w in waits[1:]:
                extra = nc.sync.nop()
                extra.ins.sync_info = mybir.SyncInfo(on_wait=[w], on_update=[])
        nc.all_engine_barrier()
        assert self.sems is not None
        popped = nc._tile_sem_poison_stack.pop()
        assert popped is self._sem_poison
        nc.clear_and_free_semaphores(list(self.sems.allocated().values()))
        nc.all_engine_barrier()

    tile.TileContext._drain_and_barrier = _drain_and_barrier
    tile.TileContext._drain_split_patched = True


def _split_excess_waits(nc, mybir, max_waits=1):
    """walrus's per-instruction sync-wait slots are tiny in this container;
    move all but the first wait of any instruction onto same-engine NOPs
    inserted immediately before it (engine stalls at the NOP instead)."""
    ctr = [0]
    for fn in nc.m.functions:
        for blk in fn.blocks:
            insts = list(blk.instructions)
            out, changed = [], False
            for inst in insts:
                si = getattr(inst, "sync_info", None)
                if si is not None and si.on_wait and len(si.on_wait) > max_waits:
                    waits = list(si.on_wait)
                    for w in waits[:-max_waits]:
                        ctr[0] += 1
                        nop = mybir.InstNoOp(
                            name=f"nopw-{ctr[0]}", ins=[], outs=[],
                            engine=inst.engine)
                        nop.sync_info = mybir.SyncInfo(on_wait=[w], on_update=[])
                        out.append(nop)
                    si.on_wait = waits[-max_waits:]
                    changed = True
                out.append(inst)
            if changed:
                blk.instructions[:] = out


def _build():
    global _built
    if _built is not None:
        return _built

    _patch_tile_drain()
    import concourse.bass as bass
    import concourse.mybir as mybir
    import concourse.tile as tile

    F32 = mybir.dt.float32
    F32R = mybir.dt.float32r
    BF16 = mybir.dt.bfloat16

    nc = bass.Bass("TRN2")
    xT = nc.dram_tensor("xT", [D, S], BF16, kind="ExternalInput")
    wqT = nc.dram_tensor("wqT", [D, C], BF16, kind="ExternalInput")
    wkT = nc.dram_tensor("wkT", [D, C], BF16, kind="ExternalInput")
    wvT = nc.dram_tensor("wvT", [D, C], BF16, kind="ExternalInput")
    woT = nc.dram_tensor("woT", [C, D], BF16, kind="ExternalInput")
    mask = nc.dram_tensor("mask", [512, 512], BF16, kind="ExternalInput")
    onesr = nc.dram_tensor("onesr", [2, 128], F32R, kind="ExternalInput")
    out = nc.dram_tensor("out", [S, D], BF16, kind="ExternalOutput")

    with tile.TileContext(nc) as tc:
        _emit(nc, tc, bass, mybir, xT, wqT, wkT, wvT, woT, mask, onesr, out,
              F32, F32R, BF16)

    _split_excess_waits(nc, mybir)
    _built = nc
    return nc


def _emit(nc, tc, bass, mybir, xT, wqT, wkT, wvT, woT, mask, onesr, out,
          F32, F32R, BF16):
    from contextlib import ExitStack

    Exp = mybir.ActivationFunctionType.Exp

    with ExitStack() as ctx:
        pers = ctx.enter_context(tc.tile_pool(name="pers", bufs=1))
        # PSUM budget (8 banks): ps_s 2x[128,1024] = 4 (scores only),
        # ps_o 3x[128,512] = 3 (op pair + rb ring), ps_f 1x[128,512] = 1
        # (all filler/proj/outproj chunks).
        ps_s = ctx.enter_context(tc.tile_pool(name="ps_s", bufs=2, space="PSUM"))
        ps_o = ctx.enter_context(tc.tile_pool(name="ps_o", bufs=3, space="PSUM"))
        ps_f = ctx.enter_context(tc.tile_pool(name="ps_f", bufs=1, space="PSUM"))
        espool = ctx.enter_context(tc.tile_pool(name="espool", bufs=12))
        small = ctx.enter_context(tc.tile_pool(name="small", bufs=10))
        outp = ctx.enter_context(tc.tile_pool(name="outp", bufs=4))

        # persistent SBUF tensors
        qt = [pers.tile([128, S], BF16, name=f"qt{i}", tag=f"qt{i}") for i in range(4)]
        kt = [pers.tile([128, S], BF16, name=f"kt{i}", tag=f"kt{i}") for i in range(4)]
        at = [pers.tile([128, S], BF16, name=f"at{i}", tag=f"at{i}") for i in range(4)]
        vt = [pers.tile([128, HL, DK + 1], BF16, name=f"vt{i}", tag=f"vt{i}")
              for i in range(16)]
        maskt = pers.tile([128, 4, 512], BF16, name="maskt", tag="maskt")
        onesA = pers.tile([1, 128], F32R, name="onesA", tag="onesA")
        onesB = pers.tile([1, 128], F32R, name="onesB", tag="onesB")
        wot = pers.tile([128, 4, D], BF16, name="wot", tag="wot")
        wq_t = pers.tile([128, 8, C], BF16, name="wq_t", tag="wq")
        wk_t = pers.tile([128, 8, C], BF16, name="wk_t", tag="wk")
        wv_t = pers.tile([128, 8, C], BF16, name="wv_t", tag="wv")
        x_t = [pers.tile([128, 8, 1024], BF16, name=f"x{i}", tag=f"x{i}")
               for i in range(2)]

        xT_r = xT[:, :].rearrange("(a p) s -> p a s", p=128)

        # startup DMAs: spread issue over engine queues; x0+wq first (they
        # gate the first matmul), x1/wot/mask behind them.
        nc.sync.dma_start(out=x_t[0], in_=xT_r[:, :, 0:1024])
        nc.sync.dma_start(out=wq_t, in_=wqT[:, :].rearrange("(a p) c -> p a c", p=128))
        nc.sync.dma_start(out=wk_t, in_=wkT[:, :].rearrange("(a p) c -> p a c", p=128))
        nc.sync.dma_start(out=wv_t, in_=wvT[:, :].rearrange("(a p) c -> p a c", p=128))
        nc.sync.dma_start(out=onesA, in_=onesr[0:1, :])
        nc.sync.dma_start(out=onesB, in_=onesr[1:2, :])
        for i in range(16):
            nc.vector.memset(vt[i][:, :, DK:DK + 1], 1.0)

        def qk_chunk(w_t, dst, cc, sb2, j, cast_vec):
            # dst[cc][:, s0:s0+512] = (W chunk) @ x chunk, 8 accum MMs N=512
            s0 = sb2 * 1024 + j * 512
            ps = ps_f.tile([128, 512], F32, name="ps_qk", tag="f")
            for dc in range(8):
                nc.tensor.matmul(
                    ps,
                    lhsT=w_t[:, dc, cc * 128:(cc + 1) * 128],
                    rhs=x_t[sb2][:, dc, j * 512:(j + 1) * 512],
                    start=(dc == 0), stop=(dc == 7))
            if cast_vec:
                nc.vector.tensor_copy(dst[cc][:, s0:s0 + 512], ps)
            else:
                nc.scalar.copy(dst[cc][:, s0:s0 + 512], ps)

        def v_chunk(si, cast_vec):
            # vt[si] = x row-block @ Wv, scattered into per-head cols
            sb2, ss = si // 8, si % 8
            ps = ps_f.tile([128, 512], F32, name="ps_v", tag="f")
            for dc in range(8):
                nc.tensor.matmul(
                    ps,
                    lhsT=x_t[sb2][:, dc, ss * 128:(ss + 1) * 128],
                    rhs=wv_t[:, dc, :],
                    start=(dc == 0), stop=(dc == 7))
            src = ps[:, :].rearrange("p (h j) -> p h j", h=HL)
            if cast_vec:
                nc.vector.tensor_copy(vt[si][:, :, 0:DK], src)
            else:
                nc.scalar.copy(vt[si][:, :, 0:DK], src)

        def outproj_chunk(qb, ss):
            # partial out rows = A^T.T @ Wo^T for one 128-query block
            r0 = qb * 512 + ss * 128
            ot = outp.tile([128, 1024], BF16, name="ot", tag="ot")
            for eb in range(2):
                pp = ps_f.tile([128, 512], F32, name="pp", tag="f")
                for cci in range(4):
                    nc.tensor.matmul(
                        pp,
                        lhsT=at[cci][:, r0:r0 + 128],
                        rhs=wot[:, cci, eb * 512:(eb + 1) * 512],
                        start=(cci == 0), stop=(cci == 3))
                nc.vector.tensor_copy(ot[:, eb * 512:(eb + 1) * 512], pp)
            nc.sync.dma_start(out=out[r0:r0 + 128, :], in_=ot)

        def _normalize(p):
            # A^T = O^T * (1/Z): fast-approx reciprocal of the Z rows, then a
            # col-tiled pair of K=1 matmuls broadcasts 1/Z over 64 partitions
            # per head into one [128,512] PSUM tile.
            cc, qb, opA, opB = p
            q0 = qb * 512
            # copy the Z rows to SBUF (f32r), broadcast Z over 64 partitions
            # per head with a col-tiled K=1 matmul pair, then one fast-approx
            # reciprocal over all 128 partitions gives 1/Z directly in SBUF.
            zA = small.tile([1, 512], F32R, name="zA", tag="r1")
            zB = small.tile([1, 512], F32R, name="zB", tag="r1")
            with nc.allow_low_precision(reason="f32r Z for PE broadcast"):
                nc.vector.tensor_copy(zA[:, :], opA[64:65, :])
                nc.vector.tensor_copy(zB[:, :], opB[64:65, :])
            rb = ps_o.tile([128, 512], F32, name="rb", tag="o")
            # rb rows 0:64 = Z_A (onesA = 1 on cols 0:64), rows 64:128 = Z_B
            # (onesB = 1 on cols 64:128), via two accumulating K=1 matmuls.
            nc.tensor.matmul(rb, lhsT=onesA[:, :], rhs=zA[:, :],
                             start=True, stop=False)
            nc.tensor.matmul(rb, lhsT=onesB[:, :], rhs=zB[:, :],
                             start=False, stop=True)
            # hand-rolled fast reciprocal (magic-constant seed + one
            # Newton step; ~0.1% rel err, plenty for the bf16 A^T).  Produces
            # -1/Z; the sign is folded into the host-negated Wo.
            I32 = mybir.dt.int32
            sd = small.tile([128, 512], F32, name="sd", tag="rbs")
            tt = small.tile([128, 512], F32, name="tt", tag="rbs")
            rbs = small.tile([128, 512], F32, name="rbs", tag="rbs")
            nc.vector.tensor_scalar(
                sd[:, :].bitcast(I32), rb[:, :].bitcast(I32),
                0x7EF311C3, -1, mybir.AluOpType.subtract, mybir.AluOpType.mult)
            nc.vector.tensor_mul(tt[:, :], rb[:, :], sd[:, :])
            nc.vector.scalar_tensor_tensor(
                rbs[:, :], tt[:, :], 2.0, sd[:, :],
                mybir.AluOpType.subtract, mybir.AluOpType.mult)
            nc.vector.tensor_mul(at[cc][0:64, q0:q0 + 512], opA[0:64, :],
                                 rbs[0:64, :])
            nc.vector.tensor_mul(at[cc][64:128, q0:q0 + 512], opB[0:64, :],
                                 rbs[64:128, :])

        pending = [None]

        def attn_pair(qb, cc, fillers, npop):
            # heads hA=2cc (partitions 0:64), hB=2cc+1 (partitions 64:128).
            # One sp/es tile per key block kb holds BOTH heads (A in half 0,
            # B in half 1) -> 2-deep ACT lookahead within 4 PSUM banks.
            q0 = qb * 512
            nkb = 4 * (qb + 1)
            hA, hB = 2 * cc, 2 * cc + 1
            es = []
            opA = opB = None

            def scores(kb):
                sp = ps_s.tile([128, 1024], F32, name="sp", tag="s")
                nc.tensor.matmul(
                    sp[:, 0:512],
                    lhsT=kt[cc][0:64, kb * 128:(kb + 1) * 128],
                    rhs=qt[cc][0:64, q0:q0 + 512], start=True, stop=True)
                nc.tensor.matmul(
                    sp[:, 512:1024],
                    lhsT=kt[cc][64:128, kb * 128:(kb + 1) * 128],
                    rhs=qt[cc][64:128, q0:q0 + 512], start=True, stop=True)
                ea = espool.tile([128, 2, 512], BF16, name="ea", tag="es")
                nc.scalar.activation(out=ea[:, :, :], in_=sp, func=Exp,
                                     scale=0.125)
                r = kb - (nkb - 4)
                if r >= 0:
                    # on GpSimd (idle; SBUF-only op) so the DVE FIFO
                    # (reciprocals, casts) never gates the PV matmuls
                    w = 128 * (r + 1)  # only queries < w can be masked
                    nc.gpsimd.tensor_mul(ea[:, 0, 0:w], ea[:, 0, 0:w],
                                         maskt[:, r, 0:w])
                    nc.gpsimd.tensor_mul(ea[:, 1, 0:w], ea[:, 1, 0:w],
                                         maskt[:, r, 0:w])
                es.append(ea)

            def pv(kb):
                nc.tensor.matmul(opA, lhsT=vt[kb][:, hA, :],
                                 rhs=es[kb][:, 0, :],
                                 start=(kb == 0), stop=(kb == nkb - 1))
                nc.tensor.matmul(opB, lhsT=vt[kb][:, hB, :],
                                 rhs=es[kb][:, 1, :],
                                 start=(kb == 0), stop=(kb == nkb - 1))

            popped = [0]

            def pop1():
                # emit one filler chunk while the ACT queue is full so the
                # PE filler work overlaps exp instead of stalling it
                if popped[0] < npop and fillers:
                    fillers.popleft()()
                    popped[0] += 1

            scores(0)
            if pending[0] is not None:
                _normalize(pending[0])
                pending[0] = None
            opA = ps_o.tile([65, 512], F32, name="opA", tag="o")
            opB = ps_o.tile([65, 512], F32, name="opB", tag="o")
            scores(1)
            pop1()
            pop1()
            for kb in range(2, nkb):
                scores(kb)
                pv(kb - 2)
                if kb >= 4 and kb % 2 == 0:
                    pop1()
            pv(nkb - 2)
            pv(nkb - 1)
            pending[0] = (cc, qb, opA, opB)
            while popped[0] < npop and fillers:
                fillers.popleft()()
                popped[0] += 1

        # ---- phase: proj sb2=0 head ----
        # only the chunks qb0's first pair needs run up front; the rest are
        # fillers inside qb0.  Late input DMAs are issued from inside this
        # stream (scalar FIFO) so x0/wq/wk/wv get full DMA bandwidth first.
        nchunk = [0]

        def late_dmas():
            nchunk[0] += 1
            if nchunk[0] == 8:
                nc.scalar.dma_start(
                    out=maskt,
                    in_=mask[:, :].rearrange("(r p) q -> p r q", p=128))
            elif nchunk[0] == 12:
                nc.scalar.dma_start(out=x_t[1], in_=xT_r[:, :, 1024:2048])
            elif nchunk[0] == 16:
                nc.scalar.dma_start(
                    out=wot,
                    in_=woT[:, :].rearrange("(a p) e -> p a e", p=128))

        for w_t, dst in ((wq_t, qt), (wk_t, kt)):
            for cc in range(4):
                for j in (0, 1):
                    qk_chunk(w_t, dst, cc, 0, j, cast_vec=False)
                    late_dmas()
        for si in range(8):
            v_chunk(si, cast_vec=False)

        # filler chunks: proj sb2=1 split so qb2 inputs (queries/keys
        # 1024:1536, V blocks 8-11) are ready before qb2, rest before qb3.
        fillers = deque()
        for w_t, dst in ((wq_t, qt), (wk_t, kt)):
            for cc in range(4):
                fillers.append(
                    lambda w_t=w_t, dst=dst, cc=cc: qk_chunk(w_t, dst, cc, 1, 0, True))
        for si in range(8, 12):
            fillers.append(lambda si=si: v_chunk(si, True))

        # ---- attention qb0 ----
        for cc in range(4):
            attn_pair(0, cc, fillers, 2)
        for ss in range(4):
            fillers.append(lambda ss=ss: outproj_chunk(0, ss))
        for cc in range(4):
            attn_pair(1, cc, fillers, 2)
        while fillers:  # safety: pre-qb2 chunks must be emitted by now
            fillers.popleft()()

        for w_t, dst in ((wq_t, qt), (wk_t, kt)):
            for cc in range(4):
                fillers.append(
                    lambda w_t=w_t, dst=dst, cc=cc: qk_chunk(w_t, dst, cc, 1, 1, True))
        for si in range(12, 16):
            fillers.append(lambda si=si: v_chunk(si, True))
        for ss in range(4):
            fillers.append(lambda ss=ss: outproj_chunk(1, ss))

        # ---- attention qb2 ----
        for cc in range(4):
            attn_pair(2, cc, fillers, 4)
        while fillers:
            fillers.popleft()()

        for ss in range(4):
            fillers.append(lambda ss=ss: outproj_chunk(2, ss))

        # ---- attention qb3 ----
        for cc in range(4):
            attn_pair(3, cc, fillers, 1)
        while fillers:
            fillers.popleft()()

        _normalize(pending[0])
        pending[0] = None
        for ss in range(4):
            outproj_chunk(3, ss)


def _prep_in_maps(x, Wq, Wk, Wv, Wo):
    import ml_dtypes

    bf = ml_dtypes.bfloat16
    x = np.asarray(x, np.float32)
    Wq = np.asarray(Wq, np.float32)
    Wk = np.asarray(Wk, np.float32)
    Wv = np.asarray(Wv, np.float32)
    Wo = np.asarray(Wo, np.float32)

    m = (np.arange(512)[:, None] <= np.arange(512)[None, :])
    mask_np = np.ascontiguousarray(m.astype(bf))

    in_maps = []
    for core in range(NCORES):
        b, g = core // 2, core % 2
        sl = slice(g * C, (g + 1) * C)
        in_maps.append({
            "xT": np.ascontiguousarray(x[b].T.astype(bf)),
            "wqT": np.ascontiguousarray(Wq[sl, :].T.astype(bf)),
            "wkT": np.ascontiguousarray(Wk[sl, :].T.astype(bf)),
            "wvT": np.ascontiguousarray(Wv[sl, :].T.astype(bf)),
            "woT": np.ascontiguousarray((-Wo[:, sl]).T.astype(bf)),
            "mask": mask_np,
            "onesr": np.repeat(np.eye(2, dtype=np.float32), 64, axis=1),
        })
    return in_maps


def _run(x, Wq, Wk, Wv, Wo, trace=False):
    from concourse.bass_utils import run_bass_kernel_spmd

    nc = _build()
    in_maps = _prep_in_maps(x, Wq, Wk, Wv, Wo)
    res = run_bass_kernel_spmd(nc, in_maps, core_ids=list(range(NCORES)),
                               trace=trace)
    full = np.empty((B, S, D), np.float32)
    for b in range(B):
        full[b] = (res.results[2 * b]["out"].astype(np.float32)
                   + res.results[2 * b + 1]["out"].astype(np.float32))
    return full, res


def kernel(x, Wq, Wk, Wv, Wo):
    full, _ = _run(x, Wq, Wk, Wv, Wo, trace=False)
    return full
